# revision 4
# baseline (speedup 1.0000x reference)
"""Cross-attention Bass kernel for Trainium2, 8 NeuronCores.

Sharding (hardcoded for B=4, Sq=Skv=2048, 16 heads, dim_head=64):
  core = 2*b + h  (b in 0..3 batches, h in 0..1 head-halves)
  - data parallel over batch B (4-way)
  - tensor parallel over heads (2-way): each core owns 8 heads = 512 of the
    1024 inner columns (to_q/k/v column-parallel, to_out row-parallel)
  - to_out partial sums are combined with an on-device ReduceScatter over
    core pairs {2b, 2b+1} at quarter-chunk (128-row) granularity in bf16;
    each core returns half of the rows of out[b] (bf16, host casts to f32).

The host pre-transposes x/context per batch (xT = x[b].T) so the kernel's
matmuls get the contraction dim on partitions without on-chip transposes.
All operands are bf16 (fp32 accumulation in PSUM).

Pipeline: the attention j-loop is ACT(exp)-bound (~1.1us per j-step).  The
q-projection of chunk sc+1 and the out-projection of chunk sc-1 are spread
as "riders", one PE matmul per j-step, inside chunk sc's attention, and each
128-row quarter of the out-projection triggers its ReduceScatter mid-chunk.
Softmax normalization is DMA-free: the denominator row (PV ones-column) is
reciprocal'ed on one DVE lane and broadcast across 64 partitions with a
K=1 PE matmul.
"""

import sys

for _p in ("/opt/trn_rl_repo",):
    if _p not in sys.path:
        sys.path.insert(0, _p)

from contextlib import ExitStack

import numpy as np

import concourse.bass as bass
import concourse.mybir as mybir
import concourse.tile as tile
from concourse import bacc
from concourse.bass import ts

F32 = mybir.dt.float32
BF16 = mybir.dt.bfloat16

# full-size problem constants
HEADS = 16
DIM_HEAD = 64
QUERY_DIM = 1024
CONTEXT_DIM = 768
INNER = HEADS * DIM_HEAD  # 1024
B_FULL, SQ_FULL, O_FULL = 4, 2048, 1024
N_CORES = 8


def build_nc(S=2048, C=1024, CK=768, I=512, O=1024, SC=512, n_cores=8):
    """Build the per-core SPMD Bass program.

    S: q/kv sequence length, C: query dim, CK: context dim,
    I: per-core inner size (heads_per_core * 64), O: output dim,
    SC: s-chunk width used as matmul moving size (<=512 for fp32 psum).
    """
    D = 64
    n_pairs = I // 128            # head pairs per core
    CT, CKT = C // 128, CK // 128
    NSC = S // SC                 # q chunks
    NJ = S // 128                 # kv blocks
    NSB = SC // 128               # s-blocks per chunk
    NOC = O // 512                # out column chunks
    NH = I // 64                  # heads per core
    scale = D ** -0.5
    groups = [[2 * i, 2 * i + 1] for i in range(n_cores // 2)]

    nc = bacc.Bacc("TRN2", target_bir_lowering=False, debug=False,
                   num_devices=n_cores)

    xT = nc.dram_tensor("xT", [C, S], BF16, kind="ExternalInput").ap()
    ctxT = nc.dram_tensor("ctxT", [CK, S], BF16, kind="ExternalInput").ap()
    wq = nc.dram_tensor("wq", [C, I], BF16, kind="ExternalInput").ap()
    wk = nc.dram_tensor("wk", [CK, I], BF16, kind="ExternalInput").ap()
    wv = nc.dram_tensor("wv", [CK, I], BF16, kind="ExternalInput").ap()
    wo = nc.dram_tensor("wo", [I, O], BF16, kind="ExternalInput").ap()
    bo = nc.dram_tensor("bo", [1, O], F32, kind="ExternalInput").ap()
    out_ext = nc.dram_tensor("out", [S // 2, O], BF16,
                             kind="ExternalOutput").ap()

    with tile.TileContext(nc) as tc, ExitStack() as stk:
        dram = stk.enter_context(tc.tile_pool(name="dram", bufs=1, space="DRAM"))
        rs_in = dram.tile([S, O], BF16, tag="rs_in")
        rs_out = [
            dram.tile([SC // 2, O], BF16, tag=f"rs_out{i}", name=f"rs_out{i}")
            for i in range(NSC)
        ]

        persist = stk.enter_context(tc.tile_pool(name="persist", bufs=1))
        qT = [persist.tile([128, S], BF16, tag=f"qT{p}", name=f"qT{p}")
              for p in range(n_pairs)]
        kT = [persist.tile([128, S], BF16, tag=f"kT{p}", name=f"kT{p}")
              for p in range(n_pairs)]
        # v augmented with a per-head ones column (65 cols/head): the PV
        # matmul emits the softmax denominator as psum row 64 for free.
        v_sb = [persist.tile([128, NH * 65], BF16, tag=f"v{j}", name=f"v{j}")
                for j in range(NJ)]
        wo_sb = [persist.tile([128, O], BF16, tag=f"wo{p}", name=f"wo{p}")
                 for p in range(n_pairs)]
        wq_sb = [persist.tile([128, I], BF16, tag=f"wq{c}", name=f"wq{c}")
                 for c in range(CT)]
        bias_sb = persist.tile([128, O], F32, tag="bias", name="bias_sb")
        ones_f32 = persist.tile([128, NH], F32, tag="ones_f", name="ones_f32")
        # row 64 holds the ones used as K=1 stationary for the denominator
        # broadcast matmul (matches the partition of the psum ones-row).
        ones_bc = persist.tile([65, 64], BF16, tag="ones_bc", name="ones_bc")

        nc.vector.memset(ones_f32[:], 1.0)
        nc.vector.memset(ones_bc[:], 1.0)

        inq = stk.enter_context(tc.tile_pool(name="inq", bufs=2))

        def qproj_prefetch(sc):
            chunk = [inq.tile([128, SC], BF16, tag=f"cq{c}", name=f"cq{c}")
                     for c in range(CT)]
            for c in range(CT):
                nc.sync.dma_start(out=chunk[c][:],
                                  in_=xT[ts(c, 128), ts(sc, SC)])
            return chunk

        # ---------------- projections: k & v from ctxT ----------------
        with ExitStack() as pstk:
            wpool = pstk.enter_context(tc.tile_pool(name="wkv", bufs=1))
            inp = pstk.enter_context(tc.tile_pool(name="inkv", bufs=1))
            psum = pstk.enter_context(
                tc.tile_pool(name="pskv", bufs=4, space="PSUM"))
            wk_sb = [wpool.tile([128, I], BF16, tag=f"wk{c}", name=f"wk{c}")
                     for c in range(CKT)]
            wv_sb = [wpool.tile([128, I], BF16, tag=f"wv{c}", name=f"wv{c}")
                     for c in range(CKT)]
            # prefetch everything needed by the head phase up front; the
            # DMA queue drains in FIFO order while the PE computes.
            for c in range(CKT):
                nc.sync.dma_start(out=wk_sb[c][:], in_=wk[ts(c, 128), :])
                nc.sync.dma_start(out=wv_sb[c][:], in_=wv[ts(c, 128), :])
            chunks = {}
            for sc in range(NSC):
                for c in range(CKT):
                    t = inp.tile([128, SC], BF16, tag=f"ckv{sc}_{c}",
                                 name=f"ckv{sc}_{c}")
                    nc.sync.dma_start(out=t[:],
                                      in_=ctxT[ts(c, 128), ts(sc, SC)])
                    chunks[sc, c] = t
                if sc == 0:
                    # attention-phase inputs stream behind the kv inputs
                    for c in range(CT):
                        nc.sync.dma_start(out=wq_sb[c][:],
                                          in_=wq[ts(c, 128), :])
                    for p in range(n_pairs):
                        nc.sync.dma_start(out=wo_sb[p][:],
                                          in_=wo[ts(p, 128), :])
                    nc.sync.dma_start(out=bias_sb[:],
                                      in_=bo.to_broadcast((128, O)))
                    q_chunk0 = qproj_prefetch(0)

            for sc in range(NSC):
                # kT[p][:, sc*SC:...] = (wk[:, p-slab].T @ ctxT[:, chunk])
                for p in range(n_pairs):
                    acc = psum.tile([128, SC], F32, tag="pkv", name="acc_kv")
                    for c in range(CKT):
                        nc.tensor.matmul(
                            acc[:], wk_sb[c][:, ts(p, 128)],
                            chunks[sc, c][:],
                            start=(c == 0), stop=(c == CKT - 1))
                    nc.vector.tensor_copy(kT[p][:, ts(sc, SC)], acc[:])
                # v rows for this chunk: v[jb] = ctxT_chunk.T @ wv
                IC = min(I, 512)
                for jb in range(NSB):
                    j = sc * NSB + jb
                    for ic in range(I // IC):
                        acc = psum.tile([128, IC], F32, tag="pkv",
                                        name="acc_v")
                        for c in range(CKT):
                            nc.tensor.matmul(
                                acc[:], chunks[sc, c][:, ts(jb, 128)],
                                wv_sb[c][:, ts(ic, IC)],
                                start=(c == 0), stop=(c == CKT - 1))
                        nh_c = IC // 64  # heads covered by this chunk
                        v_view = v_sb[j][:].rearrange(
                            "p (h e) -> p h e", e=65)
                        nc.vector.tensor_copy(
                            v_view[:, ic * nh_c:(ic + 1) * nh_c, 0:64],
                            acc[:].rearrange("p (h d) -> p h d", d=64))
                        nc.vector.tensor_copy(
                            v_view[:, ic * nh_c:(ic + 1) * nh_c, 64:65],
                            ones_f32[:, 0:nh_c].rearrange(
                                "p (h o) -> p h o", o=1))

        # ---------------- attention + q-proj + output projection --------
        with ExitStack() as astk:
            ps_sim = astk.enter_context(
                tc.tile_pool(name="ps_sim", bufs=2, space="PSUM"))
            ps_oT = astk.enter_context(
                tc.tile_pool(name="ps_oT", bufs=2, space="PSUM"))
            ps_acc = astk.enter_context(
                tc.tile_pool(name="ps_acc", bufs=1, space="PSUM"))
            epool = astk.enter_context(tc.tile_pool(name="epool", bufs=4))
            opool = astk.enter_context(tc.tile_pool(name="opool", bufs=8))
            npool = astk.enter_context(tc.tile_pool(name="npool", bufs=4))
            outp = astk.enter_context(tc.tile_pool(name="outp", bufs=4))

            v_view = [v_sb[j][:].rearrange("p (h e) -> p h e", e=65)
                      for j in range(NJ)]

            def qproj_steps(sc, chunk):
                """q-projection of chunk sc as 32 single-matmul steps."""
                box = {}

                def mk(p, c):
                    def emit():
                        if c == 0:
                            box[p] = ps_acc.tile([128, SC], F32, tag="acc",
                                                 name="acc_q")
                        nc.tensor.matmul(
                            box[p][:], wq_sb[c][:, ts(p, 128)],
                            chunk[c][:], start=(c == 0), stop=(c == CT - 1))
                        if c == CT - 1:
                            nc.vector.tensor_copy(qT[p][:, ts(sc, SC)],
                                                  box[p][:])
                    return emit

                return [mk(p, c) for p in range(n_pairs) for c in range(CT)]

            def rs_quarter(sc, qc):
                nc.gpsimd.collective_compute(
                    "ReduceScatter", mybir.AluOpType.add,
                    replica_groups=groups,
                    ins=[rs_in[sc * SC + qc * 128:
                               sc * SC + (qc + 1) * 128, :]],
                    outs=[rs_out[sc][qc * 64:(qc + 1) * 64, :]])
                nc.gpsimd.dma_start(
                    out=out_ext[sc * (SC // 2) + qc * 64:
                                sc * (SC // 2) + (qc + 1) * 64, :],
                    in_=rs_out[sc][qc * 64:(qc + 1) * 64, :])

            def outproj_steps(sc, oT_chunk):
                """out-projection of chunk sc as 32 single-matmul steps;
                each 128-row quarter fires its ReduceScatter when done."""
                box = {}

                def mk(sb, oc, p):
                    def emit():
                        if p == 0:
                            box[sb, oc] = ps_acc.tile([128, 512], F32,
                                                      tag="acc", name="acc_o")
                        acc = box[sb, oc]
                        nc.tensor.matmul(
                            acc[:], oT_chunk[p][:, ts(sb, 128)],
                            wo_sb[p][:, ts(oc, 512)],
                            start=(p == 0), stop=(p == n_pairs - 1))
                        if p == n_pairs - 1:
                            o_out = outp.tile([128, 512], BF16, tag="o_out",
                                              name="o_out")
                            nc.vector.tensor_add(o_out[:], acc[:],
                                                 bias_sb[:, ts(oc, 512)])
                            nc.sync.dma_start(
                                out=rs_in[sc * SC + sb * 128:
                                          sc * SC + sb * 128 + 128,
                                          ts(oc, 512)],
                                in_=o_out[:])
                            if oc == NOC - 1:
                                rs_quarter(sc, sb)
                    return emit

                return [mk(sb, oc, p) for sb in range(NSB)
                        for oc in range(NOC) for p in range(n_pairs)]

            def attention_chunk(sc, riders=()):
                riders = list(riders)
                oT_chunk = {}
                for p in range(n_pairs):
                    # per-head PV accumulators: rows 0..63 = oT, row 64 =
                    # the softmax denominator from the ones column of v
                    oT_ps = [ps_oT.tile([128, SC], F32, tag="oT",
                                        name=f"oT_ps{h}") for h in range(2)]
                    for j in range(NJ):
                        if riders:
                            r = riders.pop(0)
                            if r is not None:
                                r()
                        sim = ps_sim.tile([128, 2 * SC], F32, tag="sim",
                                          name="sim")
                        for h in range(2):  # head within pair
                            nc.tensor.matmul(
                                sim[:, ts(h, SC)],
                                kT[p][ts(h, 64), ts(j, 128)],
                                qT[p][ts(h, 64), ts(sc, SC)],
                                start=True, stop=True)
                        e = epool.tile([128, 2 * SC], BF16, tag="E", name="E")
                        nc.scalar.activation(
                            e[:], sim[:],
                            mybir.ActivationFunctionType.Exp, scale=scale)
                        first, last = (j == 0), (j == NJ - 1)
                        for h in range(2):
                            nc.tensor.matmul(
                                oT_ps[h][0:65, :],
                                v_view[j][:, 2 * p + h, :],
                                e[:, ts(h, SC)],
                                start=first, stop=last)
                    # normalization, DMA-free: reciprocal of the denominator
                    # row on its DVE lane, broadcast over 64 partitions with
                    # a K=1 PE matmul, multiply in place.
                    o_sb = opool.tile([128, SC], BF16, tag="oT_sb",
                                      name="oT_sb")
                    for h in range(2):
                        ou = npool.tile([65, SC], F32, tag="ou", bufs=6,
                                        name="ou")
                        nc.vector.tensor_copy(ou[:], oT_ps[h][0:65, :])
                        rec = npool.tile([65, SC], BF16, tag="rec", bufs=4,
                                         name="rec")
                        with nc.allow_low_precision(
                                reason="bf16 1/denominator, |rel|<2^-9"):
                            nc.vector.reciprocal(rec[64:65, :],
                                                 ou[64:65, :])
                        bc = ps_acc.tile([64, SC], F32, tag="bc", name="bc")
                        nc.tensor.matmul(bc[:], ones_bc[64:65, :],
                                         rec[64:65, :], start=True, stop=True)
                        if h == 0:
                            nc.vector.tensor_mul(o_sb[0:64, :],
                                                 ou[0:64, :], bc[:])
                        else:
                            # DVE lanes are partition-locked; normalize in
                            # place, DMA-shift rows into the pair slab
                            tb = npool.tile([64, SC], BF16, tag="tb",
                                            name="tb")
                            nc.vector.tensor_mul(tb[:], ou[0:64, :], bc[:])
                            nc.gpsimd.dma_start(out=o_sb[64:128, :],
                                                in_=tb[:])
                    oT_chunk[p] = o_sb
                return oT_chunk

            # software pipeline:
            #  - q-projection of chunk sc+1 and out-projection of chunk sc-1
            #    ride one matmul per j-step inside the ACT-bound attention
            #    of chunk sc; outproj riders sit in the middle so the last
            #    pair's normalization of chunk sc-1 has time to complete.
            for fn in qproj_steps(0, q_chunk0):
                fn()
            oT_prev = None
            for sc in range(NSC):
                if sc + 1 < NSC:
                    q_next = qproj_steps(sc + 1, qproj_prefetch(sc + 1))
                else:
                    q_next = []
                o_steps = outproj_steps(sc - 1, oT_prev) if sc > 0 else []
                if q_next and o_steps:
                    riders = q_next[:16] + o_steps + q_next[16:]
                elif q_next:
                    riders = q_next
                else:
                    riders = [None] * 12 + o_steps
                oT_prev = attention_chunk(sc, riders)
            # tail: out-projection of the last chunk as a burst, quarter
            # ReduceScatters pipelining behind it
            for fn in outproj_steps(NSC - 1, oT_prev):
                fn()

    nc.compile()
    return nc


# ---------------------------------------------------------------------------
# host-side sharding / unsharding
# ---------------------------------------------------------------------------

def make_in_maps(x, context, w_q, w_k, w_v, w_o, b_o, n_cores=N_CORES):
    x = np.asarray(x, dtype=np.float32)
    context = np.asarray(context, dtype=np.float32)
    w_q = np.asarray(w_q, dtype=np.float32)
    w_k = np.asarray(w_k, dtype=np.float32)
    w_v = np.asarray(w_v, dtype=np.float32)
    w_o = np.asarray(w_o, dtype=np.float32)
    b_o = np.asarray(b_o, dtype=np.float32)
    inner = w_q.shape[1]
    ih = inner // 2  # per-core inner half
    zeros_b = np.zeros_like(b_o)
    import ml_dtypes
    bf16 = ml_dtypes.bfloat16
    in_maps = []
    for core in range(n_cores):
        b, hh = core // 2, core % 2
        i0 = hh * ih
        in_maps.append({
            "xT": np.ascontiguousarray(x[b].T).astype(bf16),
            "ctxT": np.ascontiguousarray(context[b].T).astype(bf16),
            "wq": np.ascontiguousarray(w_q[:, i0:i0 + ih]).astype(bf16),
            "wk": np.ascontiguousarray(w_k[:, i0:i0 + ih]).astype(bf16),
            "wv": np.ascontiguousarray(w_v[:, i0:i0 + ih]).astype(bf16),
            "wo": np.ascontiguousarray(w_o[i0:i0 + ih, :]).astype(bf16),
            "bo": (b_o if hh == 0 else zeros_b).reshape(1, -1).copy(),
        })
    return in_maps


def gather_out(results, S, O, SC, n_cores=N_CORES):
    """Assemble full output from per-core quarter-ReduceScatter shards.

    RS runs per quarter-chunk (128 global rows): shard rows
    sc*SC/2 + qc*64 .. of core (b, hh) hold global rows
    sc*SC + qc*128 + hh*64 .. of out[b].
    """
    B = n_cores // 2
    out = np.empty((B, S, O), dtype=np.float32)
    nsc = S // SC
    nq = SC // 128   # quarters per chunk
    for core in range(n_cores):
        b, hh = core // 2, core % 2
        res = np.asarray(results[core]["out"], dtype=np.float32)  # [S//2, O]
        for c in range(nsc):
            for qc in range(nq):
                rows = res[c * (SC // 2) + qc * 64:
                           c * (SC // 2) + (qc + 1) * 64]
                g0 = c * SC + qc * 128 + hh * 64
                out[b, g0:g0 + 64, :] = rows
    return out


_NC_CACHE = {}


def _get_nc():
    if "full" not in _NC_CACHE:
        _NC_CACHE["full"] = build_nc()
    return _NC_CACHE["full"]


def kernel(x, context, w_q, w_k, w_v, w_o, b_o):
    from concourse.bass_utils import run_bass_kernel_spmd

    nc = _get_nc()
    in_maps = make_in_maps(x, context, w_q, w_k, w_v, w_o, b_o)
    res = run_bass_kernel_spmd(nc, in_maps, list(range(N_CORES)))
    return gather_out(res.results, SQ_FULL, O_FULL, 512)


# revision 5
# speedup vs baseline: 1.1370x; 1.1370x over previous
"""Cross-attention Bass kernel for Trainium2, 8 NeuronCores.

Sharding (hardcoded for B=4, Sq=Skv=2048, 16 heads, dim_head=64):
  core = 2*b + h  (b in 0..3 batches, h in 0..1 head-halves)
  - data parallel over batch B (4-way)
  - tensor parallel over heads (2-way): each core owns 8 heads = 512 of the
    1024 inner columns (to_q/k/v column-parallel, to_out row-parallel)
  - to_out partial sums are combined with an on-device ReduceScatter over
    core pairs {2b, 2b+1} at quarter-chunk (128-row) granularity in bf16;
    each core returns half of the rows of out[b] (bf16, host casts to f32).

The host pre-transposes x/context per batch (xT = x[b].T) so the kernel's
matmuls get the contraction dim on partitions without on-chip transposes.
All operands are bf16 (fp32 accumulation in PSUM).

Pipeline: the attention j-loop is ACT(exp)-bound (~1.1us per j-step).
Everything else rides inside it, at most a couple of PE ops per j-step:
  - q-projection of chunk sc+1 (steps 0-15 and 48-63 of chunk sc)
  - out-projection of chunk sc-1 (steps 16-47), each 128-row quarter
    firing its ReduceScatter as soon as its rows are in DRAM
  - softmax normalization of pair p (deferred into pair p+1's steps so the
    3.3us single-lane reciprocal never blocks the in-order PE queue):
    PSUM drain at +0, DVE reciprocal of the denominator row at +1, K=1 PE
    broadcast matmul + multiply at +8/+9.
"""

import sys

for _p in ("/opt/trn_rl_repo",):
    if _p not in sys.path:
        sys.path.insert(0, _p)

from contextlib import ExitStack

import numpy as np

import concourse.bass as bass
import concourse.mybir as mybir
import concourse.tile as tile
from concourse import bacc
from concourse.bass import ts

F32 = mybir.dt.float32
BF16 = mybir.dt.bfloat16

# full-size problem constants
HEADS = 16
DIM_HEAD = 64
QUERY_DIM = 1024
CONTEXT_DIM = 768
INNER = HEADS * DIM_HEAD  # 1024
B_FULL, SQ_FULL, O_FULL = 4, 2048, 1024
N_CORES = 8


def build_nc(S=2048, C=1024, CK=768, I=512, O=1024, SC=512, n_cores=8):
    """Build the per-core SPMD Bass program.

    S: q/kv sequence length, C: query dim, CK: context dim,
    I: per-core inner size (heads_per_core * 64), O: output dim,
    SC: s-chunk width used as matmul moving size (<=512 for fp32 psum).
    """
    D = 64
    n_pairs = I // 128            # head pairs per core
    CT, CKT = C // 128, CK // 128
    NSC = S // SC                 # q chunks
    NJ = S // 128                 # kv blocks
    NSB = SC // 128               # s-blocks per chunk
    NOC = O // 512                # out column chunks
    NH = I // 64                  # heads per core
    scale = D ** -0.5
    groups = [[2 * i, 2 * i + 1] for i in range(n_cores // 2)]

    nc = bacc.Bacc("TRN2", target_bir_lowering=False, debug=False,
                   num_devices=n_cores)

    xT = nc.dram_tensor("xT", [C, S], BF16, kind="ExternalInput").ap()
    ctxT = nc.dram_tensor("ctxT", [CK, S], BF16, kind="ExternalInput").ap()
    wq = nc.dram_tensor("wq", [C, I], BF16, kind="ExternalInput").ap()
    wk = nc.dram_tensor("wk", [CK, I], BF16, kind="ExternalInput").ap()
    wv = nc.dram_tensor("wv", [CK, I], BF16, kind="ExternalInput").ap()
    wo = nc.dram_tensor("wo", [I, O], BF16, kind="ExternalInput").ap()
    bo = nc.dram_tensor("bo", [1, O], F32, kind="ExternalInput").ap()
    out_ext = nc.dram_tensor("out", [S // 2, O], BF16,
                             kind="ExternalOutput").ap()

    with tile.TileContext(nc) as tc, ExitStack() as stk:
        dram = stk.enter_context(tc.tile_pool(name="dram", bufs=1, space="DRAM"))
        rs_in = dram.tile([S, O], BF16, tag="rs_in")
        rs_out = [
            dram.tile([SC // 2, O], BF16, tag=f"rs_out{i}", name=f"rs_out{i}")
            for i in range(NSC)
        ]

        persist = stk.enter_context(tc.tile_pool(name="persist", bufs=1))
        qT = [persist.tile([128, S], BF16, tag=f"qT{p}", name=f"qT{p}")
              for p in range(n_pairs)]
        kT = [persist.tile([128, S], BF16, tag=f"kT{p}", name=f"kT{p}")
              for p in range(n_pairs)]
        # v augmented with a per-head ones column (65 cols/head): the PV
        # matmul emits the softmax denominator as psum row 64 for free.
        v_sb = [persist.tile([128, NH * 65], BF16, tag=f"v{j}", name=f"v{j}")
                for j in range(NJ)]
        wo_sb = [persist.tile([128, O], BF16, tag=f"wo{p}", name=f"wo{p}")
                 for p in range(n_pairs)]
        wq_sb = [persist.tile([128, I], BF16, tag=f"wq{c}", name=f"wq{c}")
                 for c in range(CT)]
        # full-length input slabs: one DMA each (a dma_start costs ~650ns
        # of descriptor generation on the queue, so batch big)
        xT_sb = [persist.tile([128, S], BF16, tag=f"xs{c}", name=f"xs{c}")
                 for c in range(CT)]
        ctx_sb = [persist.tile([128, S], BF16, tag=f"cs{c}", name=f"cs{c}")
                  for c in range(CKT)]
        wk_sb = [persist.tile([128, I], BF16, tag=f"wk{c}", name=f"wk{c}")
                 for c in range(CKT)]
        wv_sb = [persist.tile([128, I], BF16, tag=f"wv{c}", name=f"wv{c}")
                 for c in range(CKT)]
        bias_sb = persist.tile([128, O], F32, tag="bias", name="bias_sb")
        ones_f32 = persist.tile([128, NH], F32, tag="ones_f", name="ones_f32")
        # row 64 holds the ones used as K=1 stationary for the denominator
        # broadcast matmul (matches the partition of the psum ones-row).
        ones_bc = persist.tile([65, 64], BF16, tag="ones_bc", name="ones_bc")

        nc.vector.memset(ones_f32[:], 1.0)
        nc.vector.memset(ones_bc[:], 1.0)

        # ---------------- input prefetch, kv-proj-first order ------------
        nc.sync.dma_start(out=wk_sb[0][:], in_=wk[ts(0, 128), :])
        nc.sync.dma_start(out=wv_sb[0][:], in_=wv[ts(0, 128), :])
        for c in range(CKT):
            nc.sync.dma_start(out=ctx_sb[c][:], in_=ctxT[ts(c, 128), :])
        for c in range(1, CKT):
            nc.sync.dma_start(out=wk_sb[c][:], in_=wk[ts(c, 128), :])
            nc.sync.dma_start(out=wv_sb[c][:], in_=wv[ts(c, 128), :])
        for c in range(CT):
            nc.sync.dma_start(out=wq_sb[c][:], in_=wq[ts(c, 128), :])
        for c in range(CT):
            nc.sync.dma_start(out=xT_sb[c][:], in_=xT[ts(c, 128), :])
        for p in range(n_pairs):
            nc.sync.dma_start(out=wo_sb[p][:], in_=wo[ts(p, 128), :])
        nc.sync.dma_start(out=bias_sb[:], in_=bo.to_broadcast((128, O)))

        # ---------------- projections: k & v from ctxT ----------------
        with ExitStack() as pstk:
            psum = pstk.enter_context(
                tc.tile_pool(name="pskv", bufs=4, space="PSUM"))
            for sc in range(NSC):
                # kT[p][:, sc*SC:...] = (wk[:, p-slab].T @ ctxT[:, chunk])
                for p in range(n_pairs):
                    acc = psum.tile([128, SC], F32, tag="pkv", name="acc_kv")
                    for c in range(CKT):
                        nc.tensor.matmul(
                            acc[:], wk_sb[c][:, ts(p, 128)],
                            ctx_sb[c][:, ts(sc, SC)],
                            start=(c == 0), stop=(c == CKT - 1))
                    nc.vector.tensor_copy(kT[p][:, ts(sc, SC)], acc[:])
                # v rows for this chunk: v[jb] = ctxT_chunk.T @ wv
                IC = min(I, 512)
                for jb in range(NSB):
                    j = sc * NSB + jb
                    for ic in range(I // IC):
                        acc = psum.tile([128, IC], F32, tag="pkv",
                                        name="acc_v")
                        for c in range(CKT):
                            nc.tensor.matmul(
                                acc[:], ctx_sb[c][:, ts(j, 128)],
                                wv_sb[c][:, ts(ic, IC)],
                                start=(c == 0), stop=(c == CKT - 1))
                        nh_c = IC // 64  # heads covered by this chunk
                        v_view = v_sb[j][:].rearrange(
                            "p (h e) -> p h e", e=65)
                        nc.vector.tensor_copy(
                            v_view[:, ic * nh_c:(ic + 1) * nh_c, 0:64],
                            acc[:].rearrange("p (h d) -> p h d", d=64))
                        nc.vector.tensor_copy(
                            v_view[:, ic * nh_c:(ic + 1) * nh_c, 64:65],
                            ones_f32[:, 0:nh_c].rearrange(
                                "p (h o) -> p h o", o=1))

        # ---------------- attention + q-proj + output projection --------
        with ExitStack() as astk:
            ps_sim = astk.enter_context(
                tc.tile_pool(name="ps_sim", bufs=2, space="PSUM"))
            ps_oT = astk.enter_context(
                tc.tile_pool(name="ps_oT", bufs=2, space="PSUM"))
            ps_acc = astk.enter_context(
                tc.tile_pool(name="ps_acc", bufs=1, space="PSUM"))
            epool = astk.enter_context(tc.tile_pool(name="epool", bufs=4))
            opool = astk.enter_context(tc.tile_pool(name="opool", bufs=8))
            npool = astk.enter_context(tc.tile_pool(name="npool", bufs=4))
            outp = astk.enter_context(tc.tile_pool(name="outp", bufs=4))

            v_view = [v_sb[j][:].rearrange("p (h e) -> p h e", e=65)
                      for j in range(NJ)]

            def qproj_steps(sc):
                """q-projection of chunk sc as 32 single-matmul steps."""
                box = {}

                def mk(p, c):
                    def emit():
                        if c == 0:
                            box[p] = ps_acc.tile([128, SC], F32, tag="acc",
                                                 name="acc_q")
                        nc.tensor.matmul(
                            box[p][:], wq_sb[c][:, ts(p, 128)],
                            xT_sb[c][:, ts(sc, SC)],
                            start=(c == 0), stop=(c == CT - 1))
                        if c == CT - 1:
                            nc.vector.tensor_copy(qT[p][:, ts(sc, SC)],
                                                  box[p][:])
                    return emit

                return [mk(p, c) for p in range(n_pairs) for c in range(CT)]

            def rs_quarter(sc, qc, out_dmas=None):
                nc.gpsimd.collective_compute(
                    "ReduceScatter", mybir.AluOpType.add,
                    replica_groups=groups,
                    ins=[rs_in[sc * SC + qc * 128:
                               sc * SC + (qc + 1) * 128, :]],
                    outs=[rs_out[sc][qc * 64:(qc + 1) * 64, :]])

                def out_dma():
                    nc.gpsimd.dma_start(
                        out=out_ext[sc * (SC // 2) + qc * 64:
                                    sc * (SC // 2) + (qc + 1) * 64, :],
                        in_=rs_out[sc][qc * 64:(qc + 1) * 64, :])

                if out_dmas is None:
                    out_dma()
                else:
                    out_dmas.append(out_dma)

            def outproj_steps(sc, oT_chunk, pool, out_dmas=None):
                """out-projection of chunk sc as 32 single-matmul steps;
                each 128-row quarter fires its ReduceScatter when done."""
                box = {}

                def mk(sb, oc, p):
                    def emit():
                        if p == 0:
                            box[sb, oc] = pool.tile(
                                [128, 512], F32,
                                tag="oT" if pool is ps_oT else "acc",
                                name="acc_o")
                        acc = box[sb, oc]
                        nc.tensor.matmul(
                            acc[:], oT_chunk[p][:, ts(sb, 128)],
                            wo_sb[p][:, ts(oc, 512)],
                            start=(p == 0), stop=(p == n_pairs - 1))
                        if p == n_pairs - 1:
                            o_out = outp.tile([128, 512], BF16, tag="o_out",
                                              name="o_out")
                            nc.vector.tensor_add(o_out[:], acc[:],
                                                 bias_sb[:, ts(oc, 512)])
                            nc.sync.dma_start(
                                out=rs_in[sc * SC + sb * 128:
                                          sc * SC + sb * 128 + 128,
                                          ts(oc, 512)],
                                in_=o_out[:])
                            if oc == NOC - 1:
                                rs_quarter(sc, sb, out_dmas)
                    return emit

                return [mk(sb, oc, p) for sb in range(NSB)
                        for oc in range(NOC) for p in range(n_pairs)]

            def norm_plan(oT_ps):
                """Normalization of one pair as 4 deferred rider steps."""
                o_sb = opool.tile([128, SC], BF16, tag="oT_sb", name="oT_sb")
                st = {}

                def s0():
                    st['ou'] = [npool.tile([65, SC], F32, tag="ou", bufs=6,
                                           name="ou") for _ in range(2)]
                    for h in range(2):
                        nc.vector.tensor_copy(st['ou'][h][:],
                                              oT_ps[h][0:65, :])

                def s1():
                    st['rec'] = [npool.tile([65, SC], BF16, tag="rec",
                                            bufs=4, name="rec")
                                 for _ in range(2)]
                    with nc.allow_low_precision(
                            reason="bf16 1/denominator, |rel|<2^-9"):
                        for h in range(2):
                            nc.vector.reciprocal(st['rec'][h][64:65, :],
                                                 st['ou'][h][64:65, :])

                def s2():
                    bc = ps_acc.tile([64, SC], F32, tag="bc", name="bc")
                    nc.tensor.matmul(bc[:], ones_bc[64:65, :],
                                     st['rec'][0][64:65, :],
                                     start=True, stop=True)
                    nc.vector.tensor_mul(o_sb[0:64, :], st['ou'][0][0:64, :],
                                         bc[:])

                def s3():
                    bc = ps_acc.tile([64, SC], F32, tag="bc", name="bc")
                    nc.tensor.matmul(bc[:], ones_bc[64:65, :],
                                     st['rec'][1][64:65, :],
                                     start=True, stop=True)
                    # DVE lanes are partition-locked; normalize in place,
                    # DMA-shift rows into the pair slab
                    tb = npool.tile([64, SC], BF16, tag="tb", name="tb")
                    nc.vector.tensor_mul(tb[:], st['ou'][1][0:64, :], bc[:])
                    nc.sync.dma_start(out=o_sb[64:128, :], in_=tb[:])

                return [s0, s1, s2, s3], o_sb

            NORM_OFFS = (0, 1, 8, 9)

            def attention_chunk(sc, slots, pending_norm):
                """Run one q-chunk; returns (oT_chunk, last pair's norm
                steps to be scheduled by the caller)."""
                for off, fn in zip(NORM_OFFS, pending_norm):
                    slots[off].append(fn)
                oT_chunk = {}
                norm_last = None
                for p in range(n_pairs):
                    # per-head PV accumulators: rows 0..63 = oT, row 64 =
                    # the softmax denominator from the ones column of v
                    oT_ps = [ps_oT.tile([128, SC], F32, tag="oT",
                                        name=f"oT_ps{h}") for h in range(2)]
                    for j in range(NJ):
                        for fn in slots[p * NJ + j]:
                            fn()
                        sim = ps_sim.tile([128, 2 * SC], F32, tag="sim",
                                          name="sim")
                        for h in range(2):  # head within pair
                            nc.tensor.matmul(
                                sim[:, ts(h, SC)],
                                kT[p][ts(h, 64), ts(j, 128)],
                                qT[p][ts(h, 64), ts(sc, SC)],
                                start=True, stop=True)
                        e = epool.tile([128, 2 * SC], BF16, tag="E", name="E")
                        nc.scalar.activation(
                            e[:], sim[:],
                            mybir.ActivationFunctionType.Exp, scale=scale)
                        first, last = (j == 0), (j == NJ - 1)
                        for h in range(2):
                            nc.tensor.matmul(
                                oT_ps[h][0:65, :],
                                v_view[j][:, 2 * p + h, :],
                                e[:, ts(h, SC)],
                                start=first, stop=last)
                    steps, o_sb = norm_plan(oT_ps)
                    if p + 1 < n_pairs:
                        base = (p + 1) * NJ
                        for off, fn in zip(NORM_OFFS, steps):
                            slots[base + off].append(fn)
                    else:
                        norm_last = steps
                    oT_chunk[p] = o_sb
                return oT_chunk, norm_last

            def make_slots(seq, offset=0):
                slots = [[] for _ in range(n_pairs * NJ)]
                for k, fn in enumerate(seq):
                    slots[offset + k].append(fn)
                return slots

            # software pipeline (see module docstring)
            for fn in qproj_steps(0):
                fn()
            oT_prev, pending = None, []
            for sc in range(NSC):
                q_next = qproj_steps(sc + 1) if sc + 1 < NSC else []
                o_steps = (outproj_steps(sc - 1, oT_prev, ps_acc)
                           if sc > 0 else [])
                if q_next and o_steps:
                    slots = make_slots(q_next[:16] + o_steps + q_next[16:])
                elif q_next:
                    slots = make_slots(q_next)
                else:
                    slots = make_slots(o_steps, offset=12)
                oT_prev, pending = attention_chunk(sc, slots, pending)
            # tail: normalize the last pair, then out-projection of the
            # last chunk as a burst; quarter ReduceScatters pipeline
            # behind it and the output DMAs flush last so they don't
            # delay the collective triggers.
            for fn in pending:
                fn()
            out_dmas = []
            for fn in outproj_steps(NSC - 1, oT_prev, ps_oT, out_dmas):
                fn()
            for fn in out_dmas:
                fn()

    nc.compile()
    return nc


# ---------------------------------------------------------------------------
# host-side sharding / unsharding
# ---------------------------------------------------------------------------

def make_in_maps(x, context, w_q, w_k, w_v, w_o, b_o, n_cores=N_CORES):
    x = np.asarray(x, dtype=np.float32)
    context = np.asarray(context, dtype=np.float32)
    w_q = np.asarray(w_q, dtype=np.float32)
    w_k = np.asarray(w_k, dtype=np.float32)
    w_v = np.asarray(w_v, dtype=np.float32)
    w_o = np.asarray(w_o, dtype=np.float32)
    b_o = np.asarray(b_o, dtype=np.float32)
    inner = w_q.shape[1]
    ih = inner // 2  # per-core inner half
    zeros_b = np.zeros_like(b_o)
    import ml_dtypes
    bf16 = ml_dtypes.bfloat16
    in_maps = []
    for core in range(n_cores):
        b, hh = core // 2, core % 2
        i0 = hh * ih
        in_maps.append({
            "xT": np.ascontiguousarray(x[b].T).astype(bf16),
            "ctxT": np.ascontiguousarray(context[b].T).astype(bf16),
            "wq": np.ascontiguousarray(w_q[:, i0:i0 + ih]).astype(bf16),
            "wk": np.ascontiguousarray(w_k[:, i0:i0 + ih]).astype(bf16),
            "wv": np.ascontiguousarray(w_v[:, i0:i0 + ih]).astype(bf16),
            "wo": np.ascontiguousarray(w_o[i0:i0 + ih, :]).astype(bf16),
            "bo": (b_o if hh == 0 else zeros_b).reshape(1, -1).copy(),
        })
    return in_maps


def gather_out(results, S, O, SC, n_cores=N_CORES):
    """Assemble full output from per-core quarter-ReduceScatter shards.

    RS runs per quarter-chunk (128 global rows): shard rows
    sc*SC/2 + qc*64 .. of core (b, hh) hold global rows
    sc*SC + qc*128 + hh*64 .. of out[b].
    """
    B = n_cores // 2
    out = np.empty((B, S, O), dtype=np.float32)
    nsc = S // SC
    nq = SC // 128   # quarters per chunk
    for core in range(n_cores):
        b, hh = core // 2, core % 2
        res = np.asarray(results[core]["out"], dtype=np.float32)  # [S//2, O]
        for c in range(nsc):
            for qc in range(nq):
                rows = res[c * (SC // 2) + qc * 64:
                           c * (SC // 2) + (qc + 1) * 64]
                g0 = c * SC + qc * 128 + hh * 64
                out[b, g0:g0 + 64, :] = rows
    return out


_NC_CACHE = {}


def _get_nc():
    if "full" not in _NC_CACHE:
        _NC_CACHE["full"] = build_nc()
    return _NC_CACHE["full"]


def kernel(x, context, w_q, w_k, w_v, w_o, b_o):
    from concourse.bass_utils import run_bass_kernel_spmd

    nc = _get_nc()
    in_maps = make_in_maps(x, context, w_q, w_k, w_v, w_o, b_o)
    res = run_bass_kernel_spmd(nc, in_maps, list(range(N_CORES)))
    return gather_out(res.results, SQ_FULL, O_FULL, 512)


# revision 9
# speedup vs baseline: 1.1584x; 1.0188x over previous
"""Cross-attention Bass kernel for Trainium2, 8 NeuronCores.

Sharding (hardcoded for B=4, Sq=Skv=2048, 16 heads, dim_head=64):
  core = 2*b + h  (b in 0..3 batches, h in 0..1 head-halves)
  - data parallel over batch B (4-way)
  - tensor parallel over heads (2-way): each core owns 8 heads = 512 of the
    1024 inner columns (to_q/k/v column-parallel, to_out row-parallel)
  - to_out partial sums are combined with an on-device ReduceScatter over
    core pairs {2b, 2b+1} at quarter-chunk (128-row) granularity in bf16;
    each core returns half of the rows of out[b] (bf16, host casts to f32).

The host pre-transposes x/context per batch (xT = x[b].T) so the kernel's
matmuls get the contraction dim on partitions without on-chip transposes.
All operands are bf16 (fp32 accumulation in PSUM).

Pipeline: the attention j-loop is ACT(exp)-bound (~1.1us per j-step).
Everything else rides inside it, at most a couple of PE ops per j-step:
  - q-projection of chunk sc+1 (steps 0-15 and 48-63 of chunk sc)
  - out-projection of chunk sc-1 (steps 16-47), each 128-row quarter
    firing its ReduceScatter as soon as its rows are in DRAM
  - softmax normalization of pair p (deferred into pair p+1's steps so the
    3.3us single-lane reciprocal never blocks the in-order PE queue):
    PSUM drain at +0, DVE reciprocal of the denominator row at +1, K=1 PE
    broadcast matmul + multiply at +8/+9.
"""

import sys

for _p in ("/opt/trn_rl_repo",):
    if _p not in sys.path:
        sys.path.insert(0, _p)

from contextlib import ExitStack

import numpy as np

import concourse.bass as bass
import concourse.mybir as mybir
import concourse.tile as tile
from concourse import bacc
from concourse.bass import ts

F32 = mybir.dt.float32
BF16 = mybir.dt.bfloat16

# full-size problem constants
HEADS = 16
DIM_HEAD = 64
QUERY_DIM = 1024
CONTEXT_DIM = 768
INNER = HEADS * DIM_HEAD  # 1024
B_FULL, SQ_FULL, O_FULL = 4, 2048, 1024
N_CORES = 8


def build_nc(S=2048, C=1024, CK=768, I=512, O=1024, SC=512, n_cores=8):
    """Build the per-core SPMD Bass program.

    S: q/kv sequence length, C: query dim, CK: context dim,
    I: per-core inner size (heads_per_core * 64), O: output dim,
    SC: s-chunk width used as matmul moving size (<=512 for fp32 psum).
    """
    D = 64
    n_pairs = I // 128            # head pairs per core
    CT, CKT = C // 128, CK // 128
    NSC = S // SC                 # q chunks
    NJ = S // 128                 # kv blocks
    NSB = SC // 128               # s-blocks per chunk
    NOC = O // 512                # out column chunks
    NH = I // 64                  # heads per core
    scale = D ** -0.5
    groups = [[2 * i, 2 * i + 1] for i in range(n_cores // 2)]

    nc = bacc.Bacc("TRN2", target_bir_lowering=False, debug=False,
                   num_devices=n_cores)

    xT = nc.dram_tensor("xT", [C, S], BF16, kind="ExternalInput").ap()
    ctxT = nc.dram_tensor("ctxT", [CK, S], BF16, kind="ExternalInput").ap()
    wq = nc.dram_tensor("wq", [C, I], BF16, kind="ExternalInput").ap()
    wk = nc.dram_tensor("wk", [CK, I], BF16, kind="ExternalInput").ap()
    wv = nc.dram_tensor("wv", [CK, I], BF16, kind="ExternalInput").ap()
    wo = nc.dram_tensor("wo", [I, O], BF16, kind="ExternalInput").ap()
    bo = nc.dram_tensor("bo", [1, O], F32, kind="ExternalInput").ap()
    out_ext = nc.dram_tensor("out", [S // 2, O], BF16,
                             kind="ExternalOutput").ap()

    with tile.TileContext(nc) as tc, ExitStack() as stk:
        dram = stk.enter_context(tc.tile_pool(name="dram", bufs=1, space="DRAM"))
        rs_in = dram.tile([S, O], BF16, tag="rs_in")
        rs_out = [
            dram.tile([SC // 2, O], BF16, tag=f"rs_out{i}", name=f"rs_out{i}")
            for i in range(NSC)
        ]

        persist = stk.enter_context(tc.tile_pool(name="persist", bufs=1))
        qT = [persist.tile([128, S], BF16, tag=f"qT{p}", name=f"qT{p}")
              for p in range(n_pairs)]
        kT = [persist.tile([128, S], BF16, tag=f"kT{p}", name=f"kT{p}")
              for p in range(n_pairs)]
        # v augmented with a per-head ones column (65 cols/head): the PV
        # matmul emits the softmax denominator as psum row 64 for free.
        v_sb = [persist.tile([128, NH * 65], BF16, tag=f"v{j}", name=f"v{j}")
                for j in range(NJ)]
        wo_sb = [persist.tile([128, O], BF16, tag=f"wo{p}", name=f"wo{p}")
                 for p in range(n_pairs)]
        wq_sb = [persist.tile([128, I], BF16, tag=f"wq{c}", name=f"wq{c}")
                 for c in range(CT)]
        # full-length input slabs: one DMA each (a dma_start costs ~650ns
        # of descriptor generation on the queue, so batch big)
        xT_sb = [persist.tile([128, S], BF16, tag=f"xs{c}", name=f"xs{c}")
                 for c in range(CT)]
        ctx_sb = [persist.tile([128, S], BF16, tag=f"cs{c}", name=f"cs{c}")
                  for c in range(CKT)]
        wk_sb = [persist.tile([128, I], BF16, tag=f"wk{c}", name=f"wk{c}")
                 for c in range(CKT)]
        wv_sb = [persist.tile([128, I], BF16, tag=f"wv{c}", name=f"wv{c}")
                 for c in range(CKT)]
        bias_sb = persist.tile([128, O], F32, tag="bias", name="bias_sb")
        ones_f32 = persist.tile([128, NH], F32, tag="ones_f", name="ones_f32")
        # row 64 holds the ones used as K=1 stationary for the denominator
        # broadcast matmul (matches the partition of the psum ones-row).
        ones_bc = persist.tile([65, 64], BF16, tag="ones_bc", name="ones_bc")

        nc.vector.memset(ones_f32[:], 1.0)
        nc.vector.memset(ones_bc[:], 1.0)

        # ---------------- input prefetch, kv-proj-first order ------------
        nc.sync.dma_start(out=wk_sb[0][:], in_=wk[ts(0, 128), :])
        nc.sync.dma_start(out=wv_sb[0][:], in_=wv[ts(0, 128), :])
        for c in range(CKT):
            nc.sync.dma_start(out=ctx_sb[c][:], in_=ctxT[ts(c, 128), :])
        for c in range(1, CKT):
            nc.sync.dma_start(out=wk_sb[c][:], in_=wk[ts(c, 128), :])
            nc.sync.dma_start(out=wv_sb[c][:], in_=wv[ts(c, 128), :])
        for c in range(CT):
            nc.sync.dma_start(out=wq_sb[c][:], in_=wq[ts(c, 128), :])
        for c in range(CT):
            nc.sync.dma_start(out=xT_sb[c][:], in_=xT[ts(c, 128), :])
        for p in range(n_pairs):
            nc.sync.dma_start(out=wo_sb[p][:], in_=wo[ts(p, 128), :])
        nc.sync.dma_start(out=bias_sb[:], in_=bo.to_broadcast((128, O)))

        # ---------------- projections: k & v from ctxT ----------------
        with ExitStack() as pstk:
            psum = pstk.enter_context(
                tc.tile_pool(name="pskv", bufs=4, space="PSUM"))
            for sc in range(NSC):
                # kT[p][:, sc*SC:...] = (wk[:, p-slab].T @ ctxT[:, chunk])
                for p in range(n_pairs):
                    acc = psum.tile([128, SC], F32, tag="pkv", name="acc_kv")
                    for c in range(CKT):
                        nc.tensor.matmul(
                            acc[:], wk_sb[c][:, ts(p, 128)],
                            ctx_sb[c][:, ts(sc, SC)],
                            start=(c == 0), stop=(c == CKT - 1))
                    nc.vector.tensor_copy(kT[p][:, ts(sc, SC)], acc[:])
                # v rows for this chunk: v[jb] = ctxT_chunk.T @ wv
                IC = min(I, 512)
                for jb in range(NSB):
                    j = sc * NSB + jb
                    for ic in range(I // IC):
                        acc = psum.tile([128, IC], F32, tag="pkv",
                                        name="acc_v")
                        for c in range(CKT):
                            nc.tensor.matmul(
                                acc[:], ctx_sb[c][:, ts(j, 128)],
                                wv_sb[c][:, ts(ic, IC)],
                                start=(c == 0), stop=(c == CKT - 1))
                        nh_c = IC // 64  # heads covered by this chunk
                        v_view = v_sb[j][:].rearrange(
                            "p (h e) -> p h e", e=65)
                        nc.vector.tensor_copy(
                            v_view[:, ic * nh_c:(ic + 1) * nh_c, 0:64],
                            acc[:].rearrange("p (h d) -> p h d", d=64))
                        nc.vector.tensor_copy(
                            v_view[:, ic * nh_c:(ic + 1) * nh_c, 64:65],
                            ones_f32[:, 0:nh_c].rearrange(
                                "p (h o) -> p h o", o=1))

        # ---------------- attention + q-proj + output projection --------
        with ExitStack() as astk:
            ps_sim = astk.enter_context(
                tc.tile_pool(name="ps_sim", bufs=2, space="PSUM"))
            ps_oT = astk.enter_context(
                tc.tile_pool(name="ps_oT", bufs=2, space="PSUM"))
            ps_acc = astk.enter_context(
                tc.tile_pool(name="ps_acc", bufs=1, space="PSUM"))
            epool = astk.enter_context(tc.tile_pool(name="epool", bufs=4))
            opool = astk.enter_context(tc.tile_pool(name="opool", bufs=8))
            npool = astk.enter_context(tc.tile_pool(name="npool", bufs=4))
            outp = astk.enter_context(tc.tile_pool(name="outp", bufs=4))

            v_view = [v_sb[j][:].rearrange("p (h e) -> p h e", e=65)
                      for j in range(NJ)]

            def qproj_steps(sc):
                """q-projection of chunk sc as 32 single-matmul steps."""
                box = {}

                def mk(p, c):
                    def emit():
                        if c == 0:
                            box[p] = ps_acc.tile([128, SC], F32, tag="acc",
                                                 name="acc_q")
                        nc.tensor.matmul(
                            box[p][:], wq_sb[c][:, ts(p, 128)],
                            xT_sb[c][:, ts(sc, SC)],
                            start=(c == 0), stop=(c == CT - 1))
                        if c == CT - 1:
                            nc.vector.tensor_copy(qT[p][:, ts(sc, SC)],
                                                  box[p][:])
                    return emit

                return [mk(p, c) for p in range(n_pairs) for c in range(CT)]

            def rs_quarter(sc, qc, out_dmas=None):
                nc.gpsimd.collective_compute(
                    "ReduceScatter", mybir.AluOpType.add,
                    replica_groups=groups,
                    ins=[rs_in[sc * SC + qc * 128:
                               sc * SC + (qc + 1) * 128, :]],
                    outs=[rs_out[sc][qc * 64:(qc + 1) * 64, :]])

                def out_dma():
                    nc.gpsimd.dma_start(
                        out=out_ext[sc * (SC // 2) + qc * 64:
                                    sc * (SC // 2) + (qc + 1) * 64, :],
                        in_=rs_out[sc][qc * 64:(qc + 1) * 64, :])

                if out_dmas is None:
                    out_dma()
                else:
                    out_dmas.append(out_dma)

            def outproj_steps(sc, oT_chunk, pool, out_dmas=None):
                """out-projection of chunk sc as 32 single-matmul steps;
                each 128-row quarter fires its ReduceScatter when done."""
                box = {}

                def mk(sb, oc, p):
                    def emit():
                        if p == 0:
                            box[sb, oc] = pool.tile(
                                [128, 512], F32,
                                tag="oT" if pool is ps_oT else "acc",
                                name="acc_o")
                        acc = box[sb, oc]
                        nc.tensor.matmul(
                            acc[:], oT_chunk[p][:, ts(sb, 128)],
                            wo_sb[p][:, ts(oc, 512)],
                            start=(p == 0), stop=(p == n_pairs - 1))
                        if p == n_pairs - 1:
                            o_out = outp.tile([128, 512], BF16, tag="o_out",
                                              name="o_out")
                            nc.vector.tensor_add(o_out[:], acc[:],
                                                 bias_sb[:, ts(oc, 512)])
                            nc.sync.dma_start(
                                out=rs_in[sc * SC + sb * 128:
                                          sc * SC + sb * 128 + 128,
                                          ts(oc, 512)],
                                in_=o_out[:])
                            if oc == NOC - 1:
                                rs_quarter(sc, sb, out_dmas)
                    return emit

                return [mk(sb, oc, p) for sb in range(NSB)
                        for oc in range(NOC) for p in range(n_pairs)]

            def norm_plan(oT_ps):
                """Normalization of one pair as 5 deferred rider steps.
                The single-lane reciprocal takes ~3.3us, so the dependent
                K=1 broadcast matmuls sit 10+ j-steps later to keep the
                in-order PE queue from ever waiting on the DVE."""
                o_sb = opool.tile([128, SC], BF16, tag="oT_sb", name="oT_sb")
                st = {}

                def s0():
                    st['ou'] = [npool.tile([65, SC], F32, tag="ou", bufs=6,
                                           name="ou") for _ in range(2)]
                    for h in range(2):
                        nc.vector.tensor_copy(st['ou'][h][:],
                                              oT_ps[h][0:65, :])
                    st['rec'] = [npool.tile([65, SC], BF16, tag="rec",
                                            bufs=4, name="rec")
                                 for _ in range(2)]

                def mk_rec(h):
                    def s_rec():
                        with nc.allow_low_precision(
                                reason="bf16 1/denominator, |rel|<2^-9"):
                            nc.vector.reciprocal(st['rec'][h][64:65, :],
                                                 st['ou'][h][64:65, :])
                    return s_rec

                def s2():
                    bc = ps_acc.tile([64, SC], F32, tag="bc", name="bc")
                    nc.tensor.matmul(bc[:], ones_bc[64:65, :],
                                     st['rec'][0][64:65, :],
                                     start=True, stop=True)
                    nc.vector.tensor_mul(o_sb[0:64, :], st['ou'][0][0:64, :],
                                         bc[:])

                def s3():
                    bc = ps_acc.tile([64, SC], F32, tag="bc", name="bc")
                    nc.tensor.matmul(bc[:], ones_bc[64:65, :],
                                     st['rec'][1][64:65, :],
                                     start=True, stop=True)
                    # DVE lanes are partition-locked; normalize in place,
                    # DMA-shift rows into the pair slab
                    tb = npool.tile([64, SC], BF16, tag="tb", name="tb")
                    nc.vector.tensor_mul(tb[:], st['ou'][1][0:64, :], bc[:])
                    nc.sync.dma_start(out=o_sb[64:128, :], in_=tb[:])

                return [s0, mk_rec(0), mk_rec(1), s2, s3], o_sb

            NORM_OFFS = (0, 1, 2, 12, 13)

            def attention_chunk(sc, slots, pending_norm):
                """Run one q-chunk; returns (oT_chunk, last pair's norm
                steps to be scheduled by the caller)."""
                for off, fn in zip(NORM_OFFS, pending_norm):
                    slots[off].append(fn)
                oT_chunk = {}
                norm_last = None
                for p in range(n_pairs):
                    # per-head PV accumulators: rows 0..63 = oT, row 64 =
                    # the softmax denominator from the ones column of v
                    oT_ps = [ps_oT.tile([128, SC], F32, tag="oT",
                                        name=f"oT_ps{h}") for h in range(2)]
                    for j in range(NJ):
                        for fn in slots[p * NJ + j]:
                            fn()
                        sim = ps_sim.tile([128, 2 * SC], F32, tag="sim",
                                          name="sim")
                        for h in range(2):  # head within pair
                            nc.tensor.matmul(
                                sim[:, ts(h, SC)],
                                kT[p][ts(h, 64), ts(j, 128)],
                                qT[p][ts(h, 64), ts(sc, SC)],
                                start=True, stop=True)
                        e = epool.tile([128, 2 * SC], BF16, tag="E", name="E")
                        nc.scalar.activation(
                            e[:], sim[:],
                            mybir.ActivationFunctionType.Exp, scale=scale)
                        first, last = (j == 0), (j == NJ - 1)
                        for h in range(2):
                            nc.tensor.matmul(
                                oT_ps[h][0:65, :],
                                v_view[j][:, 2 * p + h, :],
                                e[:, ts(h, SC)],
                                start=first, stop=last)
                    steps, o_sb = norm_plan(oT_ps)
                    if p + 1 < n_pairs:
                        base = (p + 1) * NJ
                        for off, fn in zip(NORM_OFFS, steps):
                            slots[base + off].append(fn)
                    else:
                        norm_last = steps
                    oT_chunk[p] = o_sb
                return oT_chunk, norm_last

            def make_slots(seq, offset=0):
                slots = [[] for _ in range(n_pairs * NJ)]
                for k, fn in enumerate(seq):
                    slots[offset + k].append(fn)
                return slots

            # software pipeline (see module docstring)
            for fn in qproj_steps(0):
                fn()
            oT_prev, pending = None, []
            for sc in range(NSC):
                q_next = qproj_steps(sc + 1) if sc + 1 < NSC else []
                o_steps = (outproj_steps(sc - 1, oT_prev, ps_acc)
                           if sc > 0 else [])
                if q_next and o_steps:
                    slots = make_slots(q_next[:16] + o_steps + q_next[16:])
                elif q_next:
                    slots = make_slots(q_next)
                else:
                    slots = make_slots(o_steps, offset=12)
                oT_prev, pending = attention_chunk(sc, slots, pending)
            # tail: normalize the last pair, then out-projection of the
            # last chunk as a burst; quarter ReduceScatters pipeline
            # behind it and the output DMAs flush last so they don't
            # delay the collective triggers.
            for fn in pending:
                fn()
            out_dmas = []
            for fn in outproj_steps(NSC - 1, oT_prev, ps_oT, out_dmas):
                fn()
            for fn in out_dmas:
                fn()

    nc.compile()
    return nc


# ---------------------------------------------------------------------------
# host-side sharding / unsharding
# ---------------------------------------------------------------------------

def make_in_maps(x, context, w_q, w_k, w_v, w_o, b_o, n_cores=N_CORES):
    x = np.asarray(x, dtype=np.float32)
    context = np.asarray(context, dtype=np.float32)
    w_q = np.asarray(w_q, dtype=np.float32)
    w_k = np.asarray(w_k, dtype=np.float32)
    w_v = np.asarray(w_v, dtype=np.float32)
    w_o = np.asarray(w_o, dtype=np.float32)
    b_o = np.asarray(b_o, dtype=np.float32)
    inner = w_q.shape[1]
    ih = inner // 2  # per-core inner half
    zeros_b = np.zeros_like(b_o)
    import ml_dtypes
    bf16 = ml_dtypes.bfloat16
    in_maps = []
    for core in range(n_cores):
        b, hh = core // 2, core % 2
        i0 = hh * ih
        in_maps.append({
            "xT": np.ascontiguousarray(x[b].T).astype(bf16),
            "ctxT": np.ascontiguousarray(context[b].T).astype(bf16),
            "wq": np.ascontiguousarray(w_q[:, i0:i0 + ih]).astype(bf16),
            "wk": np.ascontiguousarray(w_k[:, i0:i0 + ih]).astype(bf16),
            "wv": np.ascontiguousarray(w_v[:, i0:i0 + ih]).astype(bf16),
            "wo": np.ascontiguousarray(w_o[i0:i0 + ih, :]).astype(bf16),
            "bo": (b_o if hh == 0 else zeros_b).reshape(1, -1).copy(),
        })
    return in_maps


def gather_out(results, S, O, SC, n_cores=N_CORES):
    """Assemble full output from per-core quarter-ReduceScatter shards.

    RS runs per quarter-chunk (128 global rows): shard rows
    sc*SC/2 + qc*64 .. of core (b, hh) hold global rows
    sc*SC + qc*128 + hh*64 .. of out[b].
    """
    B = n_cores // 2
    out = np.empty((B, S, O), dtype=np.float32)
    nsc = S // SC
    nq = SC // 128   # quarters per chunk
    for core in range(n_cores):
        b, hh = core // 2, core % 2
        res = np.asarray(results[core]["out"], dtype=np.float32)  # [S//2, O]
        for c in range(nsc):
            for qc in range(nq):
                rows = res[c * (SC // 2) + qc * 64:
                           c * (SC // 2) + (qc + 1) * 64]
                g0 = c * SC + qc * 128 + hh * 64
                out[b, g0:g0 + 64, :] = rows
    return out


_NC_CACHE = {}


def _get_nc():
    if "full" not in _NC_CACHE:
        _NC_CACHE["full"] = build_nc()
    return _NC_CACHE["full"]


def kernel(x, context, w_q, w_k, w_v, w_o, b_o):
    from concourse.bass_utils import run_bass_kernel_spmd

    nc = _get_nc()
    in_maps = make_in_maps(x, context, w_q, w_k, w_v, w_o, b_o)
    res = run_bass_kernel_spmd(nc, in_maps, list(range(N_CORES)))
    return gather_out(res.results, SQ_FULL, O_FULL, 512)
